# revision 64
# baseline (speedup 1.0000x reference)
"""Trainium2 Bass kernel for nn_DiffuRNNLayer (B=8, N=2048, D=1024).

Sharding: data-parallel over batch — one batch element per NeuronCore (8 cores).
Per-core kernel works in "layout B" ([d on partitions, n on free]) with the
input pre-transposed on the host.  Phases per core:
  warm: ~3.5us of junk DR matmuls so the PE HAM clock-gate opens (2.4 GHz)
        before the first real matmul.
  A: Q/K/V projections in fp8 DoubleRow; elu+1 via an ACT psum->bf16 copy so
     the DVE ops run on 2-byte SBUF operands; K_sum accumulated with
     DR-paired ones matmuls and kept in bf16 (it scales the attention
     denominator coherently, so fp8 there is costly).
  B: KV = Kp^T V (fp8 DR) — interleaved INTO phase C's tile loop: B is
     PE-only, C is ACT/DVE-heavy, so B's matmuls fill C's PE/DMA gaps.  Two
     groups are deferred to the start of phase D to cover its prologue.
  C: acc = dwconv''(x) + MLP(x) + tokenmixer(LN(x)) into a resident BF16 acc.
     MLP1 runs in bf16 (its fp8 error dominated the baseline error budget);
     MLP2 runs fp8 DR with w2 pre-scaled x16; conv taps stay as diagonal
     matmuls folded into the MLP2 psum group (also x16).
  D: attn numerator (fp8 DR) with C1/norm folded into Qp; LN1 (gamma folded
     into ff_w1, Rsqrt fused on ACT); FFN1 bf16; FFN2 fp8 DR (fh written fp8
     by the gelu directly, f2 pre-scaled x16); LN2; write y^T (bf16).
     The per-tile FIFO order keeps PE fed: norm(t+1), ffn1(t) dc0, rep(t+1),
     ffn1(t) dc1-7, num(t+1)+LN1 stats, ffn2(t)+LN2 stats, chained LN applies.
Host transposes x/weights in, and the output back out.
"""

import numpy as np
import ml_dtypes
from contextlib import ExitStack

import concourse.bass as bass
import concourse.bacc as bacc
import concourse.tile as tile
import concourse.mybir as mybir
from concourse.bass_utils import run_bass_kernel_spmd

F32 = mybir.dt.float32
BF16 = mybir.dt.bfloat16
FP8 = mybir.dt.float8e4
AF = mybir.ActivationFunctionType
OP = mybir.AluOpType
DR = mybir.MatmulPerfMode.DoubleRow
BF16_NP = ml_dtypes.bfloat16
FP8_NP = ml_dtypes.float8_e4m3

P = 128
D = 1024
DO = D // P  # 8 chunks of the channel dim

KV_SCALE = 0.25          # kv_sb stored as KV/4 in fp8
W16 = 16.0               # fp8 weight pre-scale for w2/f2 (and the diag taps)
C1 = float(2.0 ** 21)    # qp fold: qp * C1/norm stays O(1) for fp8

# pp param-plane indices (per-partition params, laid out [128, DO, NP])
(C0, C1i, C2, CB, T0, T1, T2, TCB1, U0, U1, U2,
 TMG, TMB, N1G, N1B, N2G, N2B, LUB1, FFB1, FFB2) = range(20)
NPARAM = 20


def build_nc(N=2048, NT=512, use_bq=False, use_bk=False, use_bv=False,
             use_tmb=False, use_n1b=False, use_n2b=False,
             use_n1g=False, use_n2g=False, debug=False):
    NTILES = N // NT
    NCH = NT // P          # 128-token chunks per tile
    TOTCH = N // P
    W = NT + 4             # phase-C tile width with +-2 halo
    W8 = NT + 8            # x8 tile width padded so dual-fp8 LDW strides are 8-aligned
    assert N % NT == 0 and NT % P == 0

    nc = bacc.Bacc(None, target_bir_lowering=False, debug=debug)

    xT_d = nc.dram_tensor("x_T", [D, N], BF16, kind="ExternalInput")
    x8_d = nc.dram_tensor("x8", [D, N], FP8, kind="ExternalInput")
    w_d = {}
    for name in ("wqT", "wkT", "wvT", "f2T"):
        w_d[name] = nc.dram_tensor(name, [D, D], FP8, kind="ExternalInput")
    for name in ("w1T", "w2T", "f1T"):
        w_d[name] = nc.dram_tensor(name, [D, D], BF16, kind="ExternalInput")
    pp_d = nc.dram_tensor("pp", [P, DO, NPARAM], F32, kind="ExternalInput")
    diags_d = nc.dram_tensor("diags", [P, 3, DO, P], FP8, kind="ExternalInput")
    diagsb_d = nc.dram_tensor("diagsb", [P, 2, DO, P], BF16, kind="ExternalInput")
    rows_d = nc.dram_tensor("rows", [1, 3 * D], BF16, kind="ExternalInput")
    yT_d = nc.dram_tensor("y_T", [D, N], BF16, kind="ExternalOutput")
    qp_sp = nc.dram_tensor("qp_sp", [D, N], FP8)
    qp_r = qp_sp.rearrange("(o p) n -> p o n", p=P)

    xT = xT_d.rearrange("(o p) n -> p o n", p=P)
    x8r = x8_d.rearrange("(o p) n -> p o n", p=P)
    wr = {k: v.rearrange("(o p) n -> p o n", p=P) for k, v in w_d.items()}
    yT = yT_d.rearrange("(o p) n -> p o n", p=P)

    with tile.TileContext(nc) as tc, ExitStack() as top:
        persist = top.enter_context(tc.tile_pool(name="persist", bufs=1))
        pp = persist.tile([P, DO, NPARAM], F32)
        rows = ones_row = ones_1p_bf = None
        if use_bq or use_bk or use_bv:
            rows = persist.tile([1, 3 * D], BF16)
            nc.sync.dma_start(rows, rows_d[:])
            ones_row = persist.tile([1, NT], BF16)
            nc.vector.memset(ones_row, 1.0)
            ones_1p_bf = persist.tile([1, P], BF16)
            nc.vector.memset(ones_1p_bf, 1.0)
        repc_row = persist.tile([1, P], BF16)
        nc.vector.memset(repc_row, C1)
        # [P, 2, 16] so the dual-fp8 LDWEIGHTS half-stride is 16B-aligned
        ones_col2_t = persist.tile([P, 2, 16], FP8)
        nc.vector.memset(ones_col2_t, 1.0)
        ones_col2 = ones_col2_t[:, :, 0:1]
        ones8 = persist.tile([P, 2, P], FP8)
        nc.vector.memset(ones8, 1.0)
        ones_one = persist.tile([1, 1], BF16)
        nc.vector.memset(ones_one, 1.0)
        ksrow_sb = persist.tile([1, D], BF16)
        onesD_bf = persist.tile([P, P], BF16)
        nc.vector.memset(onesD_bf, 1.0 / D)
        eps_ln = persist.tile([P, 1], F32)
        nc.vector.memset(eps_ln, 1e-5)
        kv_sb = persist.tile([P, DO, D], FP8)
        ksum_sb = persist.tile([P, DO, 1], BF16)
        diags = persist.tile([P, 3, DO, P], FP8)
        diagsb = persist.tile([P, 2, DO, P], BF16)
        # diags DMA first: the HAM pre-warm reads it as junk rhs.
        nc.sync.dma_start(diags, diags_d[:])
        nc.sync.dma_start(diagsb, diagsb_d[:])
        nc.sync.dma_start(pp, pp_d[:])

        def stats_mm(psum, lhs_ones, rhs3, width):
            """psum[:, j] += sum over channel chunks (bf16 ones trick)."""
            for c0 in range(0, width, 512):
                cw = min(512, width - c0)
                for kc in range(DO):
                    nc.tensor.matmul(psum[:, c0:c0 + cw], lhs_ones,
                                     rhs3[:, kc, c0:c0 + cw],
                                     start=(kc == 0), stop=(kc == DO - 1))

        def stats_mm8(psum, rhs3, width):
            """fp8 DoubleRow stats: psum[:, j] = SUM over channels."""
            for c0 in range(0, width, 512):
                cw = min(512, width - c0)
                for kp in range(0, DO, 2):
                    nc.tensor.matmul(psum[:, c0:c0 + cw], ones8[:, :, 0:P],
                                     rhs3[:, kp:kp + 2, c0:c0 + cw],
                                     start=(kp == 0), stop=(kp == DO - 2),
                                     perf_mode=DR)

        # HAM pre-warm: ~3.6us of junk DR matmuls while the input DMAs run.
        with ExitStack() as warm:
            wpsum = warm.enter_context(tc.tile_pool(name="warmps", bufs=1,
                                                    space="PSUM"))
            ps_w = wpsum.tile([P, P], F32, tag="warm")
            for _ in range(28):
                nc.tensor.matmul(ps_w, ones8, diags[:, 0, 0:2, :],
                                 start=True, stop=True, perf_mode=DR)

        # The resident acc is created up-front so it sits below everything
        # on the allocation stack (it lives until the end of phase D).
        accpool = top.enter_context(tc.tile_pool(name="accres", bufs=1))
        acc_full = accpool.tile([P, DO, N], BF16)

        # Phase-C weight/input pools created before phase A so their DMAs can
        # trickle in over A's idle DMA bandwidth (the work pools that phase A
        # needs the space for come later).
        phC = ExitStack()
        wpoolC = phC.enter_context(tc.tile_pool(name="wC", bufs=1))
        ioC = phC.enter_context(tc.tile_pool(name="ioC", bufs=2))
        ioC8 = phC.enter_context(tc.tile_pool(name="ioC8", bufs=2))

        xc_tiles = {}

        def load_xc(it):
            n0 = it * NT
            x_t = ioC.tile([P, DO, W], BF16, tag="xC", name=f"x_{it}")
            x8_t = ioC8.tile([P, DO, W8], FP8, tag="x8C", name=f"x8c_{it}")
            lo, hi = n0 - 2, n0 + NT + 2
            if lo < 0:
                nc.vector.memset(x_t[:, :, 0:2], 0.0)
                nc.sync.dma_start(x_t[:, :, 2:W], xT[:, :, 0:hi])
                nc.vector.memset(x8_t[:, :, 0:2], 0.0)
                nc.sync.dma_start(x8_t[:, :, 2:W], x8r[:, :, 0:hi])
            elif hi > N:
                nc.vector.memset(x_t[:, :, W - 2:W], 0.0)
                nc.sync.dma_start(x_t[:, :, 0:W - 2], xT[:, :, lo:N])
                nc.vector.memset(x8_t[:, :, W - 2:W], 0.0)
                nc.sync.dma_start(x8_t[:, :, 0:W - 2], x8r[:, :, lo:N])
            else:
                nc.sync.dma_start(x_t, xT[:, :, lo:hi])
                nc.sync.dma_start(x8_t[:, :, 0:W], x8r[:, :, lo:hi])
            xc_tiles[it] = (x_t, x8_t)

        # kp/v live on the RIGHT side of the SBUF heap: their lifetime
        # (phase A .. early phase D) straddles the phase-C pool scope, and
        # pool release must be LIFO per heap side.
        kvres_stack = ExitStack()
        kvres = kvres_stack.enter_context(
            tc.tile_pool(name="kvres", bufs=1, side="right"))
        kp_full = kvres.tile([P, TOTCH, D], FP8)
        v_full = kvres.tile([P, TOTCH, D], FP8)

        # ---------------- Phase A: QKV (fp8 DoubleRow) ----------------
        with ExitStack() as pha:
            wpool = pha.enter_context(tc.tile_pool(name="wA", bufs=1))
            io = pha.enter_context(tc.tile_pool(name="ioA", bufs=2))
            ev = pha.enter_context(tc.tile_pool(name="evA", bufs=2))
            wq_sb = wpool.tile([P, DO, D], FP8, tag="wq")
            # dc=0 slice first so the first Q matmul starts earlier
            nc.sync.dma_start(wq_sb[:, :, 0:P], wr["wqT"][:, :, 0:P])
            xts = {}

            def load_x8(it):
                t = io.tile([P, DO, NT], FP8, tag="xA", name=f"x8a_{it}")
                nc.sync.dma_start(t, x8r[:, :, it * NT:(it + 1) * NT])
                xts[it] = t

            load_x8(0)
            nc.sync.dma_start(wq_sb[:, :, P:D], wr["wqT"][:, :, P:D])
            wk_sb = wpool.tile([P, DO, D], FP8, tag="wk")
            nc.sync.dma_start(wk_sb, wr["wkT"])
            wv_sb = wpool.tile([P, DO, D], FP8, tag="wv")
            nc.sync.dma_start(wv_sb, wr["wvT"])
            psq_pool = pha.enter_context(tc.tile_pool(name="psAq", bufs=3, space="PSUM"))
            ps = pha.enter_context(tc.tile_pool(name="psA", bufs=3, space="PSUM"))
            ksp = pha.enter_context(tc.tile_pool(name="ksA", bufs=1, space="PSUM"))

            def elu1(ps_in, out, width, tag):
                """out = elu(ps_in)+1 = max(ps,0) + min(exp(ps),1).

                exp reads the PSUM directly (exp(min(x,0)) == min(exp(x),1)),
                so the ACT queue depends only on PE and the DVE queue only on
                ACT — no cross-engine zigzag to head-of-line block on."""
                e_t = ev.tile([P, width], BF16, tag="eA", name=f"e{tag}")
                nc.scalar.activation(e_t, ps_in, AF.Exp)
                m_t = ev.tile([P, width], BF16, tag="mA", name=f"m{tag}")
                nc.vector.tensor_scalar_min(m_t, e_t, 1.0)
                nc.vector.scalar_tensor_tensor(out, ps_in, 0.0, m_t,
                                               OP.max, OP.add)

            def q_chunk(it, x_t, dc):
                n0 = it * NT
                ps_q = psq_pool.tile([P, NT], F32, tag="psq")
                for kp in range(0, DO, 2):
                    nc.tensor.matmul(ps_q, wq_sb[:, kp:kp + 2, dc * P:(dc + 1) * P],
                                     x_t[:, kp:kp + 2, :], start=(kp == 0),
                                     stop=(kp == DO - 2 and not use_bq),
                                     perf_mode=DR)
                if use_bq:
                    nc.tensor.matmul(ps_q, rows[0:1, dc * P:(dc + 1) * P],
                                     ones_row[0:1, :], start=False, stop=True)
                qp_c = ev.tile([P, NT], FP8, tag="qpA", name=f"qp_{it}_{dc}")
                elu1(ps_q, qp_c, NT, f"q{it}_{dc}")
                nc.sync.dma_start(qp_r[:, dc, n0:n0 + NT], qp_c)

            def kv_chunk(it, x_t, ch, half):
                cg = it * NCH + ch
                cs = slice(ch * P, (ch + 1) * P)
                hs = slice(half * 512, (half + 1) * 512)
                ps_k = ps.tile([P, 512], F32, tag="pskv")
                for kp in range(0, DO, 2):
                    nc.tensor.matmul(ps_k, x_t[:, kp:kp + 2, cs],
                                     wk_sb[:, kp:kp + 2, hs],
                                     start=(kp == 0),
                                     stop=(kp == DO - 2 and not use_bk),
                                     perf_mode=DR)
                if use_bk:
                    nc.tensor.matmul(ps_k, ones_1p_bf[0:1, :],
                                     rows[0:1, D + half * 512:D + (half + 1) * 512],
                                     start=False, stop=True)
                elu1(ps_k, kp_full[:, cg, hs], 512, f"k{cg}_{half}")

                ps_v = ps.tile([P, 512], F32, tag="pskv")
                for kp in range(0, DO, 2):
                    nc.tensor.matmul(ps_v, x_t[:, kp:kp + 2, cs],
                                     wv_sb[:, kp:kp + 2, hs],
                                     start=(kp == 0),
                                     stop=(kp == DO - 2 and not use_bv),
                                     perf_mode=DR)
                if use_bv:
                    nc.tensor.matmul(ps_v, ones_1p_bf[0:1, :],
                                     rows[0:1, 2 * D + half * 512:2 * D + (half + 1) * 512],
                                     start=False, stop=True)
                nc.scalar.activation(v_full[:, cg, hs], ps_v, AF.Copy)

            for it in range(NTILES):
                if it + 1 < NTILES:
                    load_x8(it + 1)
                x_t = xts.pop(it)
                for i in range(DO):
                    q_chunk(it, x_t, i)
                    kv_chunk(it, x_t, i // 2, i % 2)
            # phase-C input DMAs issued only now: ahead of this point they
            # would delay the x8 tile loads on the same DMA queue.
            load_xc(0)
            w1_sb = wpoolC.tile([P, DO, D], BF16, tag="w1")
            nc.sync.dma_start(w1_sb, wr["w1T"])
            w2_sb = wpoolC.tile([P, DO, D], BF16, tag="w2")
            nc.sync.dma_start(w2_sb, wr["w2T"])
            # K_sum in one batch at phase-A end (a per-chunk accumulation
            # would make every ksum matmul a hard PE->DVE sync point and
            # pace the whole phase at the elu-chain latency).  One [1,512]
            # psum bank, reused for the two halves.
            for hi, hs in enumerate((slice(0, 512), slice(512, 1024))):
                ps_ks = ksp.tile([1, 512], F32, tag="ksrow", name=f"ks_{hi}")
                for cg in range(1, TOTCH, 2):
                    nc.tensor.matmul(ps_ks[0:1, :], ones_col2,
                                     kp_full[:, cg - 1:cg + 1, hs],
                                     start=(cg == 1),
                                     stop=(cg == TOTCH - 1), perf_mode=DR)
                nc.scalar.activation(ksrow_sb[0:1, hs], ps_ks[0:1, :], AF.Copy)
            # transpose K_sum row -> per-partition column layout [P, DO] bf16
            ps_ksc = ksp.tile([P, DO], F32, tag="kscol")
            for dc in range(DO):
                nc.tensor.matmul(ps_ksc[:, dc:dc + 1],
                                 ksrow_sb[0:1, dc * P:(dc + 1) * P],
                                 ones_one[0:1, 0:1], start=True, stop=True)
            nc.scalar.activation(ksum_sb[:, :, 0], ps_ksc, AF.Copy)

        # ---- phase-C work pools (allocated only now: phase A needed the
        # space these occupy) ----
        pipe = phC.enter_context(tc.tile_pool(name="pipeC", bufs=2))
        mid = phC.enter_context(tc.tile_pool(name="midC", bufs=1))
        smC = phC.enter_context(tc.tile_pool(name="smC", bufs=1))
        psC = phC.enter_context(tc.tile_pool(name="psC", bufs=2, space="PSUM"))
        pstC = phC.enter_context(tc.tile_pool(name="pstC", bufs=1, space="PSUM"))

        # ---------------- Phase C + interleaved B ----------------
        # B groups: (eh, dc) -> kv_sb[:, dc, eh*512:(eh+1)*512].  The last
        # DEFER_B groups are emitted at the start of phase D to cover its
        # prologue (DVE qp-scale) with PE work.
        b_groups = [(eh, dc) for eh in range(D // 512) for dc in range(DO)]
        DEFER_B = 2

        def b_group(eh, dc, pool, tag):
            hs = slice(eh * 512, (eh + 1) * 512)
            kv_ps = pool.tile([P, 512], F32, tag=tag, name=f"kvps_{eh}_{dc}")
            for chp in range(0, TOTCH, 2):
                nc.tensor.matmul(kv_ps,
                                 kp_full[:, chp:chp + 2, dc * P:(dc + 1) * P],
                                 v_full[:, chp:chp + 2, hs],
                                 start=(chp == 0),
                                 stop=(chp == TOTCH - 2), perf_mode=DR)
            nc.scalar.activation(kv_sb[:, dc, hs], kv_ps, AF.Copy,
                                 scale=KV_SCALE)

        with phC:
            def c_front(it):
                n0 = it * NT
                x_t, x8_t = xc_tiles.pop(it)
                acc = acc_full[:, :, n0:n0 + NT]
                # local MLP entirely in bf16 (fp8 anywhere in this branch
                # dominated the error budget).  Emitted before the acc init:
                # MLP2 gates on these gelus, so they must not sit behind an
                # ACT op that waits on the next x DMA.
                h1_t = pipe.tile([P, DO, NT], BF16, tag="h1", name=f"h1_{it}")
                for dc in range(DO):
                    ps_h = psC.tile([P, NT], F32, tag="psh", name=f"psh1_{it}_{dc}")
                    for kc in range(DO):
                        nc.tensor.matmul(ps_h, w1_sb[:, kc, dc * P:(dc + 1) * P],
                                         x_t[:, kc, 2:NT + 2],
                                         start=(kc == 0), stop=(kc == DO - 1))
                    nc.scalar.activation(h1_t[:, dc, :], ps_h, AF.Gelu,
                                         bias=pp[:, dc, LUB1:LUB1 + 1])

                # diffusion dwconv'' center tap + bias on ACT (side taps join
                # the c_back PSUM group as diagonal matmuls)
                for o in range(DO):
                    nc.scalar.activation(acc[:, o, :], x_t[:, o, 2:NT + 2],
                                         AF.Identity, bias=pp[:, o, CB:CB + 1],
                                         scale=pp[:, o, C1i:C1i + 1])

                # token mixer LN stats via fp8 DR on x8 (ones=1; /D at readout)
                sq_t = mid.tile([P, DO, W], FP8, tag="sq8", name=f"sq_{it}")
                nc.scalar.activation(sq_t, x8_t[:, :, 0:W], AF.Square)
                ps_m = pstC.tile([P, W], F32, tag="psm", name=f"psm_{it}")
                stats_mm8(ps_m, x8_t[:, :, 0:W], W)
                ps_s = pstC.tile([P, W], F32, tag="pss", name=f"pss_{it}")
                stats_mm8(ps_s, sq_t, W)
                m_sb = smC.tile([P, W], BF16, tag="msb", name=f"msb_{it}")
                nc.scalar.activation(m_sb, ps_m, AF.Copy, scale=1.0 / D)
                var = smC.tile([P, W], F32, tag="var", name=f"var_{it}")
                nc.scalar.activation(var, ps_m, AF.Square, scale=1.0 / D)
                nc.vector.scalar_tensor_tensor(var, ps_s, 1.0 / D, var,
                                               OP.mult, OP.subtract)
                nc.scalar.activation(var, var, AF.Sqrt, bias=eps_ln[:, 0:1])
                nc.vector.reciprocal_approx_fast(out=var, in_=var)
                rstd = smC.tile([P, W], BF16, tag="rstd", name=f"rstd_{it}")
                nc.vector.tensor_copy(rstd, var)
                # xm = (x - m) * rstd  (tm gamma folded into conv1 taps)
                xm_t = mid.tile([P, DO, W], BF16, tag="tokC", name=f"xm_{it}")
                nc.vector.tensor_sub(xm_t, x_t,
                                     m_sb[:, None, :].broadcast_to([P, DO, W]))
                nc.vector.tensor_mul(xm_t, xm_t,
                                     rstd[:, None, :].broadcast_to([P, DO, W]))
                if use_tmb:
                    for o in range(DO):
                        nc.vector.tensor_scalar_add(xm_t[:, o, :], xm_t[:, o, :],
                                                    pp[:, o, TMB:TMB + 1])
                # conv1: t_s[k] = conv1(xm)[k+1], k in [0, W-2)
                t_t = mid.tile([P, DO, W - 2], BF16, tag="tokD", name=f"t_{it}")
                for o in range(DO):
                    nc.scalar.activation(t_t[:, o, :], xm_t[:, o, 1:W - 1],
                                         AF.Identity, bias=pp[:, o, TCB1:TCB1 + 1],
                                         scale=pp[:, o, T1:T1 + 1])
                for o in range(DO):
                    nc.vector.scalar_tensor_tensor(t_t[:, o, :], xm_t[:, o, 0:W - 2],
                                                   pp[:, o, T0:T0 + 1],
                                                   t_t[:, o, :], OP.mult, OP.add)
                for o in range(DO):
                    nc.vector.scalar_tensor_tensor(t_t[:, o, :], xm_t[:, o, 2:W],
                                                   pp[:, o, T2:T2 + 1],
                                                   t_t[:, o, :], OP.mult, OP.add)
                t2_t = pipe.tile([P, DO, W - 2], BF16, tag="t2", name=f"t2_{it}")
                nc.scalar.activation(t2_t, t_t, AF.Gelu)
                if it == 0:
                    nc.vector.memset(t2_t[:, :, 0:1], 0.0)
                if it == NTILES - 1:
                    nc.vector.memset(t2_t[:, :, W - 3:W - 2], 0.0)
                return x_t, h1_t, t2_t

            def c_back(it, tiles):
                n0 = it * NT
                x_t, h1_t, t2_t = tiles
                for dc in range(DO):
                    # token-mixer conv2 taps on DVE (all-bf16 SBUF ops): phase
                    # C is PE-bound with DVE slack, so the three diagonal
                    # matmuls these replace were pure PE cost.
                    tm = smC.tile([P, NT], BF16, tag="tmconv", name=f"tm_{it}_{dc}")
                    nc.vector.tensor_scalar_mul(tm, t2_t[:, dc, 0:NT],
                                                pp[:, dc, U0:U0 + 1])
                    nc.vector.scalar_tensor_tensor(tm, t2_t[:, dc, 1:NT + 1],
                                                   pp[:, dc, U1:U1 + 1],
                                                   tm, OP.mult, OP.add)
                    nc.vector.scalar_tensor_tensor(tm, t2_t[:, dc, 2:NT + 2],
                                                   pp[:, dc, U2:U2 + 1],
                                                   tm, OP.mult, OP.add)
                    # own tag: MLP2's consumer (the DVE acc-add) lands late in
                    # the DVE queue; sharing slots would WAR-block MLP1/B.
                    ps_h = psC.tile([P, NT], F32, tag="psh2", name=f"psh2_{it}_{dc}")
                    for kc in range(DO):
                        nc.tensor.matmul(ps_h, w2_sb[:, kc, dc * P:(dc + 1) * P],
                                         h1_t[:, kc, :],
                                         start=(kc == 0), stop=False)
                    # diffusion conv side taps as diagonal matmuls
                    nc.tensor.matmul(ps_h, diagsb[:, 0, dc, :],
                                     x_t[:, dc, 1:NT + 1], start=False, stop=False)
                    nc.tensor.matmul(ps_h, diagsb[:, 1, dc, :],
                                     x_t[:, dc, 3:NT + 3], start=False, stop=True)
                    nc.vector.tensor_add(tm, tm, ps_h)
                    nc.vector.tensor_add(acc_full[:, dc, n0:n0 + NT],
                                         acc_full[:, dc, n0:n0 + NT], tm)

            n_b = len(b_groups) - DEFER_B
            per_tile = (n_b + NTILES - 1) // NTILES
            pend = {}
            for it in range(NTILES):
                if it + 1 < NTILES:
                    load_xc(it + 1)
                for g in b_groups[it * per_tile:min((it + 1) * per_tile, n_b)]:
                    b_group(*g, psC, "psh")
                if it == 0:
                    pend[0] = c_front(0)
                if it + 1 < NTILES:
                    pend[it + 1] = c_front(it + 1)
                c_back(it, pend.pop(it))

        # ---------------- Phase D: attention + LN1 + FFN + LN2 ----------------
        with ExitStack() as ph:
            ioD = ph.enter_context(tc.tile_pool(name="ioD", bufs=3))
            qp_tiles = {}

            def load_qp(it):
                t = ioD.tile([P, DO, NT], FP8, tag="qpD", name=f"qp_{it}")
                nc.sync.dma_start(t, qp_r[:, :, it * NT:(it + 1) * NT])
                qp_tiles[it] = t

            # qp(0) queued ahead of the FFN weight DMAs so the first
            # numerator matmuls don't wait for the weights
            load_qp(0)
            wpoolD = ph.enter_context(tc.tile_pool(name="wD", bufs=1))
            f1_sb = wpoolD.tile([P, DO, D], BF16, tag="f1")
            nc.sync.dma_start(f1_sb, wr["f1T"])
            f2_sb = wpoolD.tile([P, DO, D], FP8, tag="f2")
            nc.sync.dma_start(f2_sb, wr["f2T"])
            # y1(t) overlaps y1(t+1) (bufs=2); the other D intermediates are
            # dead before their next-tile reincarnation (bufs=1).
            midD2 = ph.enter_context(tc.tile_pool(name="midD2", bufs=2))
            midD = ph.enter_context(tc.tile_pool(name="midD", bufs=1))
            sm = ph.enter_context(tc.tile_pool(name="smD", bufs=2))
            ps = ph.enter_context(tc.tile_pool(name="psD", bufs=2, space="PSUM"))
            psf_pool = ph.enter_context(tc.tile_pool(name="psfD", bufs=2, space="PSUM"))
            pst = ph.enter_context(tc.tile_pool(name="pstD", bufs=1, space="PSUM"))

            def d_norm(it):
                """norm row + reciprocal chain (DVE) for tile it."""
                if it not in qp_tiles:
                    load_qp(it)
                qp_t = qp_tiles[it]
                if it + 1 < NTILES:
                    load_qp(it + 1)
                ps_n = pst.tile([P, NT], F32, tag="psnrep", name=f"psn_{it}")
                for kc in range(DO):
                    nc.tensor.matmul(ps_n[0:1, :], ksum_sb[:, kc, :],
                                     qp_t[:, kc, :],
                                     start=(kc == 0), stop=(kc == DO - 1))
                nr = sm.tile([1, NT], F32, tag="nrD", name=f"nr_{it}")
                nc.vector.tensor_scalar_add(nr, ps_n[0:1, :], 1e-6)
                rr = sm.tile([1, NT], F32, tag="rrD", name=f"rr_{it}")
                nc.vector.reciprocal_approx_fast(out=rr, in_=nr)
                rrb = sm.tile([1, NT], BF16, tag="rrbD", name=f"rrb_{it}")
                nc.vector.tensor_copy(rrb, rr)
                return rrb

            def d_rep(it, rrb):
                """rep = C1/norm broadcast; qp *= rep (DVE, ~4us)."""
                qp_t = qp_tiles[it]
                ps_rep = pst.tile([P, NT], F32, tag="psnrep", name=f"psrep_{it}")
                nc.tensor.matmul(ps_rep, repc_row[0:1, :], rrb, start=True,
                                 stop=True)
                rep_sb = sm.tile([P, NT], BF16, tag="repsb", name=f"rep_{it}")
                nc.scalar.activation(rep_sb, ps_rep, AF.Copy)
                # the norm/rep pipeline runs two tiles ahead, so this ~4us
                # DVE op has a full iteration of slack before num(it) needs it
                nc.vector.tensor_mul(qp_t, qp_t,
                                     rep_sb[:, None, :].broadcast_to([P, DO, NT]))

            def d_num(it):
                """numerator chunks; squares on GPSIMD (SBUF->SBUF)."""
                n0 = it * NT
                qp_t = qp_tiles.pop(it)
                acc_t = acc_full[:, :, n0:n0 + NT]
                sq_t = midD.tile([P, DO, NT], FP8, tag="sqD8", name=f"sqD_{it}")
                for ec in range(DO):
                    ps_u = ps.tile([P, NT], F32, tag="psnum", name=f"psnum_{it}_{ec}")
                    for kp in range(0, DO, 2):
                        nc.tensor.matmul(ps_u, kv_sb[:, kp:kp + 2, ec * P:(ec + 1) * P],
                                         qp_t[:, kp:kp + 2, :],
                                         start=(kp == 0), stop=(kp == DO - 2),
                                         perf_mode=DR)
                    nc.vector.scalar_tensor_tensor(acc_t[:, ec, :], ps_u,
                                                   1.0 / (KV_SCALE * C1),
                                                   acc_t[:, ec, :], OP.mult, OP.add)
                    nc.scalar.activation(sq_t[:, ec, :], acc_t[:, ec, :],
                                         AF.Square)
                return acc_t, sq_t

            def d_stats(y_t, sq_t, mtag, stag, name):
                """LN stats in one batch (streamed-per-chunk stats make each
                stats matmul a hard PE->DVE/GPSIMD sync point)."""
                ps_m = pst.tile([P, NT], F32, tag=mtag, name=f"psm{name}")
                ps_s = pst.tile([P, NT], F32, tag=stag, name=f"pss{name}")
                stats_mm(ps_m, onesD_bf, y_t, NT)
                stats_mm8(ps_s, sq_t, NT)
                return ps_m, ps_s

            def ln_pre(ps_m, ps_s, kind, name):
                """mean copy + var (without the Rsqrt): same-ACT-table ops."""
                m_sb = sm.tile([P, NT], BF16, tag=f"m{kind}", name=f"m{name}")
                nc.scalar.activation(m_sb, ps_m, AF.Copy)
                var = sm.tile([P, NT], F32, tag=f"v{kind}", name=f"v{name}")
                nc.scalar.activation(var, ps_m, AF.Square)
                nc.vector.scalar_tensor_tensor(var, ps_s, 1.0 / D, var,
                                               OP.mult, OP.subtract)
                return m_sb, var

            def ln_rstd(var, kind, name):
                nc.scalar.activation(var, var, AF.Sqrt, bias=eps_ln[:, 0:1])
                nc.vector.reciprocal_approx_fast(out=var, in_=var)
                rstd = sm.tile([P, NT], BF16, tag=f"r{kind}", name=f"r{name}")
                nc.vector.tensor_copy(rstd, var)
                return rstd

            def d_y1(it, acc_t, m1_sb, rstd1):
                y1_t = midD2.tile([P, DO, NT], BF16, tag="y1", name=f"y1_{it}")
                for kc in range(DO):
                    nc.vector.tensor_sub(y1_t[:, kc, :], acc_t[:, kc, :], m1_sb)
                    nc.vector.tensor_mul(y1_t[:, kc, :], y1_t[:, kc, :], rstd1)
                    if use_n1b:
                        nc.vector.tensor_scalar_add(y1_t[:, kc, :], y1_t[:, kc, :],
                                                    pp[:, kc, N1B:N1B + 1])
                return y1_t

            def d_ffn1(it, y1_t, dcs):
                """FFN1 (bf16) for output chunks dcs; gelu writes fh fp8."""
                if it not in ffn1_fh:
                    ffn1_fh[it] = midD.tile([P, DO, NT], FP8, tag="f1h",
                                            name=f"f1h_{it}")
                fh_t = ffn1_fh[it]
                for dc in dcs:
                    ps_f = psf_pool.tile([P, NT], F32, tag="psf",
                                         name=f"psf1_{it}_{dc}")
                    for kc in range(DO):
                        nc.tensor.matmul(ps_f, f1_sb[:, kc, dc * P:(dc + 1) * P],
                                         y1_t[:, kc, :],
                                         start=(kc == 0), stop=(kc == DO - 1))
                    nc.scalar.activation(fh_t[:, dc, :], ps_f, AF.Gelu,
                                         bias=pp[:, dc, FFB1:FFB1 + 1])
                return fh_t

            def d_ffn2(it, y1_t, fh_t):
                """FFN2 in fp8 DR (f2 x16) + residual.  The psum scale runs
                on ACT and the residual add on DVE (cheap all-bf16 op);
                squares on GPSIMD."""
                y2_t = midD.tile([P, DO, NT], BF16, tag="y2", name=f"y2_{it}")
                sq2_t = midD.tile([P, DO, NT], FP8, tag="sq28", name=f"sq2_{it}")
                for dc in range(DO):
                    ps_f = psf_pool.tile([P, NT], F32, tag="psf",
                                         name=f"psf2_{it}_{dc}")
                    for kp in range(0, DO, 2):
                        nc.tensor.matmul(ps_f, f2_sb[:, kp:kp + 2, dc * P:(dc + 1) * P],
                                         fh_t[:, kp:kp + 2, :],
                                         start=(kp == 0), stop=(kp == DO - 2),
                                         perf_mode=DR)
                    y2tmp = sm.tile([P, NT], BF16, tag="y2t", name=f"y2t_{it}_{dc}")
                    nc.scalar.activation(y2tmp, ps_f, AF.Copy, scale=1.0 / W16)
                    if use_n1g:
                        # y2 = y1*g + f2(h)/16 (residual gamma re-applied)
                        nc.vector.scalar_tensor_tensor(y2_t[:, dc, :],
                                                       y1_t[:, dc, :],
                                                       pp[:, dc, N1G:N1G + 1],
                                                       y2tmp, OP.mult, OP.add)
                    else:
                        nc.vector.tensor_add(y2_t[:, dc, :], y2tmp,
                                             y1_t[:, dc, :])
                    nc.scalar.activation(sq2_t[:, dc, :], y2_t[:, dc, :],
                                         AF.Square)
                return y2_t, sq2_t

            def d_out(it, y2_t, m2_sb, rstd2):
                n0 = it * NT
                yo_t = midD.tile([P, DO, NT], BF16, tag="yo", name=f"yo_{it}")
                # last tile: two half-width DMAs so the final drain pipelines
                nsplit = 2 if it == NTILES - 1 else 1
                cw = NT // nsplit
                for ci in range(nsplit):
                    cs = slice(ci * cw, (ci + 1) * cw)
                    for dc in range(DO):
                        yo = yo_t[:, dc, cs]
                        nc.vector.tensor_sub(yo, y2_t[:, dc, cs], m2_sb[:, cs])
                        if use_n2g:
                            nc.vector.scalar_tensor_tensor(yo, yo,
                                                           pp[:, dc, N2G:N2G + 1],
                                                           rstd2[:, cs],
                                                           OP.mult, OP.mult)
                        else:
                            nc.vector.tensor_mul(yo, yo, rstd2[:, cs])
                        if use_n2b:
                            nc.vector.tensor_scalar_add(yo, yo,
                                                        pp[:, dc, N2B:N2B + 1])
                    nc.sync.dma_start(yT[:, :, n0 + ci * cw:n0 + (ci + 1) * cw],
                                      yo_t[:, :, cs])

            ffn1_fh = {}
            # Prologue: norm+rep two tiles deep (the GPSIMD qp-scale takes
            # ~8us and the numerator gates on it), deferred B groups to keep
            # PE fed meanwhile, then tile 0's numerator + LN1.
            d_rep(0, d_norm(0))
            for g in b_groups[len(b_groups) - DEFER_B:]:
                b_group(*g, ps, "psnum")
            kvres_stack.close()
            d_rep(1, d_norm(1))
            acc0, sq0 = d_num(0)
            st_m, st_s = d_stats(acc0, sq0, "psm1", "pss1", "1_0")
            m1_sb, var1 = ln_pre(st_m, st_s, "1", "1_0")
            rstd1 = ln_rstd(var1, "1", "1_0")
            y1_cur = d_y1(0, acc0, m1_sb, rstd1)
            for it in range(NTILES):
                d_ffn1(it, y1_cur, range(DO))
                Tn = d_num(it + 1) if it + 1 < NTILES else None
                y2_t, sq2_t = d_ffn2(it, y1_cur, ffn1_fh.pop(it))
                # LN chains batched by ACT table: both pre-chains (copy +
                # square), then both Sqrts, then the applies.
                if Tn is not None:
                    acc_n, sq_n = Tn
                    m1n_ps, s1n_ps = d_stats(acc_n, sq_n, "psm1", "pss1",
                                             f"1_{it + 1}")
                m2_ps, s2_ps = d_stats(y2_t, sq2_t, "psnrep", "pss2", f"2_{it}")
                if Tn is not None:
                    m1n, var1n = ln_pre(m1n_ps, s1n_ps, "1", f"1_{it + 1}")
                m2_sb, var2 = ln_pre(m2_ps, s2_ps, "2", f"2_{it}")
                if Tn is not None:
                    rstd1n = ln_rstd(var1n, "1", f"1_{it + 1}")
                    y1_cur = d_y1(it + 1, acc_n, m1n, rstd1n)
                rstd2 = ln_rstd(var2, "2", f"2_{it}")
                d_out(it, y2_t, m2_sb, rstd2)
                # norm/rep of tile it+2 emitted last: its ~4us DVE qp-scale
                # must not sit ahead of y1(it+1) in the DVE queue, and its
                # PE rows fill the tail while the LN chains drain.
                if it + 2 < NTILES:
                    d_rep(it + 2, d_norm(it + 2))

    nc.compile()
    return nc


def make_in_maps(inputs, n_cores=8):
    """Host-side preprocessing: fold constants, transpose, cast, shard."""
    x = np.asarray(inputs["x"], np.float32)
    B, N, D_ = x.shape
    dt = float(np.asarray(inputs["delta_t"]))

    def g(k):
        return np.asarray(inputs[k], np.float32)

    diff_w, diff_b = g("diff_w"), g("diff_b")
    tm_w1, tm_cb1 = g("tm_w1"), g("tm_cb1")
    tm_w2, tm_cb2 = g("tm_w2"), g("tm_cb2")
    tm_g = g("tm_g")
    n1_g, n2_g = g("n1_g"), g("n2_g")

    pp = np.zeros((P, DO, NPARAM), np.float32)

    def put(i, v):
        pp[:, :, i] = v.reshape(DO, P).T

    put(C0, dt * diff_w[:, 0, 0])
    put(C1i, dt * diff_w[:, 0, 1] + (1.0 - dt))
    put(C2, dt * diff_w[:, 0, 2])
    put(CB, dt * diff_b + g("lu_b2") + tm_cb2)
    # token-mixer gamma folded into the conv1 taps
    put(T0, tm_w1[:, 0, 0] * tm_g)
    put(T1, tm_w1[:, 0, 1] * tm_g)
    put(T2, tm_w1[:, 0, 2] * tm_g)
    put(TCB1, tm_cb1)
    put(U0, tm_w2[:, 0, 0])
    put(U1, tm_w2[:, 0, 1])
    put(U2, tm_w2[:, 0, 2])
    put(TMG, tm_g)
    put(TMB, g("tm_beta"))
    put(N1G, n1_g)
    put(N1B, g("n1_b"))
    put(N2G, n2_g)
    put(N2B, g("n2_b"))
    put(LUB1, g("lu_b1"))
    put(FFB1, g("ff_b1"))
    put(FFB2, g("ff_b2"))

    # diags (fp8): token-mixer conv2 taps; diagsb (bf16): diffusion side taps
    diags = np.zeros((P, 3, DO, P), np.float32)
    diagsb = np.zeros((P, 2, DO, P), np.float32)
    idx = np.arange(P)
    for tap in range(3):
        for dc in range(DO):
            diags[idx, tap, dc, idx] = tm_w2[dc * P + idx, 0, tap]
    for dc in range(DO):
        diagsb[idx, 0, dc, idx] = dt * diff_w[dc * P + idx, 0, 0]
        diagsb[idx, 1, dc, idx] = dt * diff_w[dc * P + idx, 0, 2]
    diags = np.clip(diags, -240, 240).astype(FP8_NP)
    diagsb = diagsb.astype(BF16_NP)

    rows = np.zeros((1, 3 * D), np.float32)
    rows[0, 0:D] = g("bq")
    rows[0, D:2 * D] = g("bk")
    rows[0, 2 * D:3 * D] = g("bv")
    rows = rows.astype(BF16_NP)

    use_n1g = bool(np.any(n1_g != 1.0))
    use_n2g = bool(np.any(n2_g != 1.0))

    wt = {}
    # The local-MLP branch stays fully bf16 (fp8 there dominated the
    # error budget)
    wt["w1T"] = np.ascontiguousarray(g("lu_w1").T).astype(BF16_NP)
    wt["w2T"] = np.ascontiguousarray(g("lu_w2").T).astype(BF16_NP)
    # n1 gamma folded into ff_w1 input rows (when gamma != 1)
    f1 = g("ff_w1")
    if use_n1g:
        f1 = f1 * n1_g[None, :]
    wt["f1T"] = np.ascontiguousarray(f1.T).astype(BF16_NP)
    # FFN2 weights shipped fp8 pre-scaled x16 (consumer applies 1/16)
    wt["f2T"] = np.ascontiguousarray(
        np.clip(g("ff_w2").T * W16, -240, 240)).astype(FP8_NP)
    for name, key in (("wqT", "wq"), ("wkT", "wk"), ("wvT", "wv")):
        wt[name] = np.ascontiguousarray(
            np.clip(g(key).T, -240, 240)).astype(FP8_NP)

    xT = np.ascontiguousarray(x.transpose(0, 2, 1)).astype(BF16_NP)
    x8 = np.clip(xT.astype(np.float32), -240, 240).astype(FP8_NP)

    flags = dict(
        use_bq=bool(np.any(g("bq"))),
        use_bk=bool(np.any(g("bk"))),
        use_bv=bool(np.any(g("bv"))),
        use_tmb=bool(np.any(g("tm_beta"))),
        use_n1b=bool(np.any(g("n1_b"))),
        use_n2b=bool(np.any(g("n2_b"))),
        use_n1g=use_n1g,
        use_n2g=use_n2g,
    )

    shared = {**wt, "pp": pp, "rows": rows, "diags": diags,
              "diagsb": diagsb}
    in_maps = [{**shared, "x_T": xT[b], "x8": x8[b]} for b in range(B)]
    return in_maps, flags, (B, N)


_NC_CACHE = {}


def kernel(**inputs):
    in_maps, flags, (B, N) = make_in_maps(inputs)
    key = (N, tuple(sorted(flags.items())))
    if key not in _NC_CACHE:
        _NC_CACHE[key] = build_nc(N=N, NT=512, **flags)
    nc = _NC_CACHE[key]
    res = run_bass_kernel_spmd(nc, in_maps, list(range(B)))
    y = np.stack([res.results[b]["y_T"] for b in range(B)])
    return np.ascontiguousarray(y.transpose(0, 2, 1)).astype(np.float32)


# revision 68
# speedup vs baseline: 1.0407x; 1.0407x over previous
"""Trainium2 Bass kernel for nn_DiffuRNNLayer (B=8, N=2048, D=1024).

Sharding: data-parallel over batch — one batch element per NeuronCore (8 cores).
Per-core kernel works in "layout B" ([d on partitions, n on free]) with the
input pre-transposed on the host.  Phases per core:
  warm: ~3.5us of junk DR matmuls so the PE HAM clock-gate opens (2.4 GHz)
        before the first real matmul.
  A: Q/K/V projections in fp8 DoubleRow; elu+1 via an ACT psum->bf16 copy so
     the DVE ops run on 2-byte SBUF operands; K_sum accumulated with
     DR-paired ones matmuls and kept in bf16 (it scales the attention
     denominator coherently, so fp8 there is costly).
  B: KV = Kp^T V (fp8 DR) — interleaved INTO phase C's tile loop: B is
     PE-only, C is ACT/DVE-heavy, so B's matmuls fill C's PE/DMA gaps.  Two
     groups are deferred to the start of phase D to cover its prologue.
  C: acc = dwconv''(x) + MLP(x) + tokenmixer(LN(x)) into a resident BF16 acc.
     MLP1 runs in bf16 (its fp8 error dominated the baseline error budget);
     MLP2 runs fp8 DR with w2 pre-scaled x16; conv taps stay as diagonal
     matmuls folded into the MLP2 psum group (also x16).
  D: attn numerator (fp8 DR) with C1/norm folded into Qp; LN1 (gamma folded
     into ff_w1, Rsqrt fused on ACT); FFN1 bf16; FFN2 fp8 DR (fh written fp8
     by the gelu directly, f2 pre-scaled x16); LN2; write y^T (bf16).
     The per-tile FIFO order keeps PE fed: norm(t+1), ffn1(t) dc0, rep(t+1),
     ffn1(t) dc1-7, num(t+1)+LN1 stats, ffn2(t)+LN2 stats, chained LN applies.
Host transposes x/weights in, and the output back out.
"""

import numpy as np
import ml_dtypes
from contextlib import ExitStack

import concourse.bass as bass
import concourse.bacc as bacc
import concourse.tile as tile
import concourse.mybir as mybir
from concourse.bass_utils import run_bass_kernel_spmd

F32 = mybir.dt.float32
BF16 = mybir.dt.bfloat16
FP8 = mybir.dt.float8e4
AF = mybir.ActivationFunctionType
OP = mybir.AluOpType
DR = mybir.MatmulPerfMode.DoubleRow
BF16_NP = ml_dtypes.bfloat16
FP8_NP = ml_dtypes.float8_e4m3

P = 128
D = 1024
DO = D // P  # 8 chunks of the channel dim

KV_SCALE = 0.25          # kv_sb stored as KV/4 in fp8
W16 = 16.0               # fp8 weight pre-scale for w2/f2 (and the diag taps)
C1 = float(2.0 ** 21)    # qp fold: qp * C1/norm stays O(1) for fp8

# pp param-plane indices (per-partition params, laid out [128, DO, NP])
(C0, C1i, C2, CB, T0, T1, T2, TCB1, U0, U1, U2,
 TMG, TMB, N1G, N1B, N2G, N2B, LUB1, FFB1, FFB2) = range(20)
NPARAM = 20


def build_nc(N=2048, NT=512, use_bq=False, use_bk=False, use_bv=False,
             use_tmb=False, use_n1b=False, use_n2b=False,
             use_n1g=False, use_n2g=False, debug=False):
    NTILES = N // NT
    NCH = NT // P          # 128-token chunks per tile
    TOTCH = N // P
    W = NT + 4             # phase-C tile width with +-2 halo
    W8 = NT + 8            # x8 tile width padded so dual-fp8 LDW strides are 8-aligned
    assert N % NT == 0 and NT % P == 0

    nc = bacc.Bacc(None, target_bir_lowering=False, debug=debug)

    xT_d = nc.dram_tensor("x_T", [D, N], BF16, kind="ExternalInput")
    x8_d = nc.dram_tensor("x8", [D, N], FP8, kind="ExternalInput")
    w_d = {}
    for name in ("wqT", "wkT", "wvT", "f2T"):
        w_d[name] = nc.dram_tensor(name, [D, D], FP8, kind="ExternalInput")
    for name in ("w1T", "w2T", "f1T"):
        w_d[name] = nc.dram_tensor(name, [D, D], BF16, kind="ExternalInput")
    pp_d = nc.dram_tensor("pp", [P, DO, NPARAM], F32, kind="ExternalInput")
    diags_d = nc.dram_tensor("diags", [P, 3, DO, P], FP8, kind="ExternalInput")
    diagsb_d = nc.dram_tensor("diagsb", [P, 2, DO, P], BF16, kind="ExternalInput")
    rows_d = nc.dram_tensor("rows", [1, 3 * D], BF16, kind="ExternalInput")
    yT_d = nc.dram_tensor("y_T", [D, N], BF16, kind="ExternalOutput")
    qp_sp = nc.dram_tensor("qp_sp", [D, N], FP8)
    qp_r = qp_sp.rearrange("(o p) n -> p o n", p=P)

    xT = xT_d.rearrange("(o p) n -> p o n", p=P)
    x8r = x8_d.rearrange("(o p) n -> p o n", p=P)
    wr = {k: v.rearrange("(o p) n -> p o n", p=P) for k, v in w_d.items()}
    yT = yT_d.rearrange("(o p) n -> p o n", p=P)

    with tile.TileContext(nc) as tc, ExitStack() as top:
        persist = top.enter_context(tc.tile_pool(name="persist", bufs=1))
        pp = persist.tile([P, DO, NPARAM], F32)
        rows = ones_row = ones_1p_bf = None
        if use_bq or use_bk or use_bv:
            rows = persist.tile([1, 3 * D], BF16)
            nc.sync.dma_start(rows, rows_d[:])
            ones_row = persist.tile([1, NT], BF16)
            nc.vector.memset(ones_row, 1.0)
            ones_1p_bf = persist.tile([1, P], BF16)
            nc.vector.memset(ones_1p_bf, 1.0)
        repc_row = persist.tile([1, P], BF16)
        nc.vector.memset(repc_row, C1)
        # [P, 2, 16] so the dual-fp8 LDWEIGHTS half-stride is 16B-aligned
        ones_col2_t = persist.tile([P, 2, 16], FP8)
        nc.vector.memset(ones_col2_t, 1.0)
        ones_col2 = ones_col2_t[:, :, 0:1]
        ones8 = persist.tile([P, 2, P], FP8)
        nc.vector.memset(ones8, 1.0)
        ones_one = persist.tile([1, 1], BF16)
        nc.vector.memset(ones_one, 1.0)
        ksrow_sb = persist.tile([1, D], BF16)
        onesD_bf = persist.tile([P, P], BF16)
        nc.vector.memset(onesD_bf, 1.0 / D)
        eps_ln = persist.tile([P, 1], F32)
        nc.vector.memset(eps_ln, 1e-5)
        kv_sb = persist.tile([P, DO, D], FP8)
        ksum_sb = persist.tile([P, DO, 1], BF16)
        diags = persist.tile([P, 3, DO, P], FP8)
        diagsb = persist.tile([P, 2, DO, P], BF16)
        # diags DMA first: the HAM pre-warm reads it as junk rhs.
        nc.sync.dma_start(diags, diags_d[:])
        nc.sync.dma_start(diagsb, diagsb_d[:])
        nc.sync.dma_start(pp, pp_d[:])

        def stats_mm(psum, lhs_ones, rhs3, width):
            """psum[:, j] += sum over channel chunks (bf16 ones trick)."""
            for c0 in range(0, width, 512):
                cw = min(512, width - c0)
                for kc in range(DO):
                    nc.tensor.matmul(psum[:, c0:c0 + cw], lhs_ones,
                                     rhs3[:, kc, c0:c0 + cw],
                                     start=(kc == 0), stop=(kc == DO - 1))

        def stats_mm8(psum, rhs3, width):
            """fp8 DoubleRow stats: psum[:, j] = SUM over channels."""
            for c0 in range(0, width, 512):
                cw = min(512, width - c0)
                for kp in range(0, DO, 2):
                    nc.tensor.matmul(psum[:, c0:c0 + cw], ones8[:, :, 0:P],
                                     rhs3[:, kp:kp + 2, c0:c0 + cw],
                                     start=(kp == 0), stop=(kp == DO - 2),
                                     perf_mode=DR)

        # HAM pre-warm: ~3.6us of junk DR matmuls while the input DMAs run.
        with ExitStack() as warm:
            wpsum = warm.enter_context(tc.tile_pool(name="warmps", bufs=1,
                                                    space="PSUM"))
            ps_w = wpsum.tile([P, P], F32, tag="warm")
            for _ in range(28):
                nc.tensor.matmul(ps_w, ones8, diags[:, 0, 0:2, :],
                                 start=True, stop=True, perf_mode=DR)

        # The resident acc is created up-front so it sits below everything
        # on the allocation stack (it lives until the end of phase D).
        accpool = top.enter_context(tc.tile_pool(name="accres", bufs=1))
        acc_full = accpool.tile([P, DO, N], BF16)

        # Phase-C weight/input pools created before phase A so their DMAs can
        # trickle in over A's idle DMA bandwidth (the work pools that phase A
        # needs the space for come later).
        phC = ExitStack()
        wpoolC = phC.enter_context(tc.tile_pool(name="wC", bufs=1))
        ioC = phC.enter_context(tc.tile_pool(name="ioC", bufs=2))
        ioC8 = phC.enter_context(tc.tile_pool(name="ioC8", bufs=2))

        xc_tiles = {}

        def load_xc(it):
            n0 = it * NT
            x_t = ioC.tile([P, DO, W], BF16, tag="xC", name=f"x_{it}")
            x8_t = ioC8.tile([P, DO, W8], FP8, tag="x8C", name=f"x8c_{it}")
            lo, hi = n0 - 2, n0 + NT + 2
            if lo < 0:
                nc.vector.memset(x_t[:, :, 0:2], 0.0)
                nc.sync.dma_start(x_t[:, :, 2:W], xT[:, :, 0:hi])
                nc.vector.memset(x8_t[:, :, 0:2], 0.0)
                nc.sync.dma_start(x8_t[:, :, 2:W], x8r[:, :, 0:hi])
            elif hi > N:
                nc.vector.memset(x_t[:, :, W - 2:W], 0.0)
                nc.sync.dma_start(x_t[:, :, 0:W - 2], xT[:, :, lo:N])
                nc.vector.memset(x8_t[:, :, W - 2:W], 0.0)
                nc.sync.dma_start(x8_t[:, :, 0:W - 2], x8r[:, :, lo:N])
            else:
                nc.sync.dma_start(x_t, xT[:, :, lo:hi])
                nc.sync.dma_start(x8_t[:, :, 0:W], x8r[:, :, lo:hi])
            xc_tiles[it] = (x_t, x8_t)

        # kp/v live on the RIGHT side of the SBUF heap: their lifetime
        # (phase A .. early phase D) straddles the phase-C pool scope, and
        # pool release must be LIFO per heap side.
        kvres_stack = ExitStack()
        kvres = kvres_stack.enter_context(
            tc.tile_pool(name="kvres", bufs=1, side="right"))
        kp_full = kvres.tile([P, TOTCH, D], FP8)
        v_full = kvres.tile([P, TOTCH, D], FP8)

        # ---------------- Phase A: QKV (fp8 DoubleRow) ----------------
        with ExitStack() as pha:
            wpool = pha.enter_context(tc.tile_pool(name="wA", bufs=1))
            io = pha.enter_context(tc.tile_pool(name="ioA", bufs=2))
            ev = pha.enter_context(tc.tile_pool(name="evA", bufs=2))
            wq_sb = wpool.tile([P, DO, D], FP8, tag="wq")
            # dc=0 slice first so the first Q matmul starts earlier
            nc.sync.dma_start(wq_sb[:, :, 0:P], wr["wqT"][:, :, 0:P])
            xts = {}

            def load_x8(it):
                t = io.tile([P, DO, NT], FP8, tag="xA", name=f"x8a_{it}")
                nc.sync.dma_start(t, x8r[:, :, it * NT:(it + 1) * NT])
                xts[it] = t

            load_x8(0)
            nc.sync.dma_start(wq_sb[:, :, P:D], wr["wqT"][:, :, P:D])
            wk_sb = wpool.tile([P, DO, D], FP8, tag="wk")
            nc.sync.dma_start(wk_sb, wr["wkT"])
            wv_sb = wpool.tile([P, DO, D], FP8, tag="wv")
            nc.sync.dma_start(wv_sb, wr["wvT"])
            psq_pool = pha.enter_context(tc.tile_pool(name="psAq", bufs=3, space="PSUM"))
            ps = pha.enter_context(tc.tile_pool(name="psA", bufs=3, space="PSUM"))
            ksp = pha.enter_context(tc.tile_pool(name="ksA", bufs=1, space="PSUM"))

            def elu1(ps_in, out, width, tag):
                """out = elu(ps_in)+1 = max(ps,0) + min(exp(ps),1).

                exp reads the PSUM directly (exp(min(x,0)) == min(exp(x),1)),
                so the ACT queue depends only on PE and the DVE queue only on
                ACT — no cross-engine zigzag to head-of-line block on."""
                e_t = ev.tile([P, width], BF16, tag="eA", name=f"e{tag}")
                nc.scalar.activation(e_t, ps_in, AF.Exp)
                m_t = ev.tile([P, width], BF16, tag="mA", name=f"m{tag}")
                nc.vector.tensor_scalar_min(m_t, e_t, 1.0)
                nc.vector.scalar_tensor_tensor(out, ps_in, 0.0, m_t,
                                               OP.max, OP.add)

            def q_chunk(it, x_t, dc):
                n0 = it * NT
                ps_q = psq_pool.tile([P, NT], F32, tag="psq")
                for kp in range(0, DO, 2):
                    nc.tensor.matmul(ps_q, wq_sb[:, kp:kp + 2, dc * P:(dc + 1) * P],
                                     x_t[:, kp:kp + 2, :], start=(kp == 0),
                                     stop=(kp == DO - 2 and not use_bq),
                                     perf_mode=DR)
                if use_bq:
                    nc.tensor.matmul(ps_q, rows[0:1, dc * P:(dc + 1) * P],
                                     ones_row[0:1, :], start=False, stop=True)
                qp_c = ev.tile([P, NT], FP8, tag="qpA", name=f"qp_{it}_{dc}")
                elu1(ps_q, qp_c, NT, f"q{it}_{dc}")
                nc.sync.dma_start(qp_r[:, dc, n0:n0 + NT], qp_c)

            def kv_chunk(it, x_t, ch, half):
                cg = it * NCH + ch
                cs = slice(ch * P, (ch + 1) * P)
                hs = slice(half * 512, (half + 1) * 512)
                ps_k = ps.tile([P, 512], F32, tag="pskv")
                for kp in range(0, DO, 2):
                    nc.tensor.matmul(ps_k, x_t[:, kp:kp + 2, cs],
                                     wk_sb[:, kp:kp + 2, hs],
                                     start=(kp == 0),
                                     stop=(kp == DO - 2 and not use_bk),
                                     perf_mode=DR)
                if use_bk:
                    nc.tensor.matmul(ps_k, ones_1p_bf[0:1, :],
                                     rows[0:1, D + half * 512:D + (half + 1) * 512],
                                     start=False, stop=True)
                elu1(ps_k, kp_full[:, cg, hs], 512, f"k{cg}_{half}")

                ps_v = ps.tile([P, 512], F32, tag="pskv")
                for kp in range(0, DO, 2):
                    nc.tensor.matmul(ps_v, x_t[:, kp:kp + 2, cs],
                                     wv_sb[:, kp:kp + 2, hs],
                                     start=(kp == 0),
                                     stop=(kp == DO - 2 and not use_bv),
                                     perf_mode=DR)
                if use_bv:
                    nc.tensor.matmul(ps_v, ones_1p_bf[0:1, :],
                                     rows[0:1, 2 * D + half * 512:2 * D + (half + 1) * 512],
                                     start=False, stop=True)
                nc.scalar.activation(v_full[:, cg, hs], ps_v, AF.Copy)

            for it in range(NTILES):
                if it + 1 < NTILES:
                    load_x8(it + 1)
                x_t = xts.pop(it)
                for i in range(DO):
                    q_chunk(it, x_t, i)
                    kv_chunk(it, x_t, i // 2, i % 2)
            # phase-C input DMAs issued only now: ahead of this point they
            # would delay the x8 tile loads on the same DMA queue.
            load_xc(0)
            w1_sb = wpoolC.tile([P, DO, D], BF16, tag="w1")
            nc.sync.dma_start(w1_sb, wr["w1T"])
            w2_sb = wpoolC.tile([P, DO, D], BF16, tag="w2")
            nc.sync.dma_start(w2_sb, wr["w2T"])
            # K_sum in one batch at phase-A end (a per-chunk accumulation
            # would make every ksum matmul a hard PE->DVE sync point and
            # pace the whole phase at the elu-chain latency).  One [1,512]
            # psum bank, reused for the two halves.
            for hi, hs in enumerate((slice(0, 512), slice(512, 1024))):
                ps_ks = ksp.tile([1, 512], F32, tag="ksrow", name=f"ks_{hi}")
                for cg in range(1, TOTCH, 2):
                    nc.tensor.matmul(ps_ks[0:1, :], ones_col2,
                                     kp_full[:, cg - 1:cg + 1, hs],
                                     start=(cg == 1),
                                     stop=(cg == TOTCH - 1), perf_mode=DR)
                nc.scalar.activation(ksrow_sb[0:1, hs], ps_ks[0:1, :], AF.Copy)
            # transpose K_sum row -> per-partition column layout [P, DO] bf16
            ps_ksc = ksp.tile([P, DO], F32, tag="kscol")
            for dc in range(DO):
                nc.tensor.matmul(ps_ksc[:, dc:dc + 1],
                                 ksrow_sb[0:1, dc * P:(dc + 1) * P],
                                 ones_one[0:1, 0:1], start=True, stop=True)
            nc.scalar.activation(ksum_sb[:, :, 0], ps_ksc, AF.Copy)

        # ---- phase-C work pools (allocated only now: phase A needed the
        # space these occupy) ----
        pipe = phC.enter_context(tc.tile_pool(name="pipeC", bufs=2))
        mid = phC.enter_context(tc.tile_pool(name="midC", bufs=1))
        smC = phC.enter_context(tc.tile_pool(name="smC", bufs=1))
        psC = phC.enter_context(tc.tile_pool(name="psC", bufs=2, space="PSUM"))
        pstC = phC.enter_context(tc.tile_pool(name="pstC", bufs=1, space="PSUM"))

        # ---------------- Phase C + interleaved B ----------------
        # B groups: (eh, dc) -> kv_sb[:, dc, eh*512:(eh+1)*512].  The last
        # DEFER_B groups are emitted at the start of phase D to cover its
        # prologue (DVE qp-scale) with PE work.
        b_groups = [(eh, dc) for eh in range(D // 512) for dc in range(DO)]
        DEFER_B = 2

        def b_group(eh, dc, pool, tag):
            hs = slice(eh * 512, (eh + 1) * 512)
            kv_ps = pool.tile([P, 512], F32, tag=tag, name=f"kvps_{eh}_{dc}")
            for chp in range(0, TOTCH, 2):
                nc.tensor.matmul(kv_ps,
                                 kp_full[:, chp:chp + 2, dc * P:(dc + 1) * P],
                                 v_full[:, chp:chp + 2, hs],
                                 start=(chp == 0),
                                 stop=(chp == TOTCH - 2), perf_mode=DR)
            nc.scalar.activation(kv_sb[:, dc, hs], kv_ps, AF.Copy,
                                 scale=KV_SCALE)

        with phC:
            def c_front(it):
                n0 = it * NT
                x_t, x8_t = xc_tiles.pop(it)
                acc = acc_full[:, :, n0:n0 + NT]
                # local MLP entirely in bf16 (fp8 anywhere in this branch
                # dominated the error budget).  Emitted before the acc init:
                # MLP2 gates on these gelus, so they must not sit behind an
                # ACT op that waits on the next x DMA.
                h1_t = pipe.tile([P, DO, NT], BF16, tag="h1", name=f"h1_{it}")
                for dc in range(DO):
                    ps_h = psC.tile([P, NT], F32, tag="psh", name=f"psh1_{it}_{dc}")
                    for kc in range(DO):
                        nc.tensor.matmul(ps_h, w1_sb[:, kc, dc * P:(dc + 1) * P],
                                         x_t[:, kc, 2:NT + 2],
                                         start=(kc == 0), stop=(kc == DO - 1))
                    nc.scalar.activation(h1_t[:, dc, :], ps_h, AF.Gelu,
                                         bias=pp[:, dc, LUB1:LUB1 + 1])

                # diffusion dwconv'' center tap + bias on ACT (side taps join
                # the c_back PSUM group as diagonal matmuls)
                for o in range(DO):
                    nc.scalar.activation(acc[:, o, :], x_t[:, o, 2:NT + 2],
                                         AF.Identity, bias=pp[:, o, CB:CB + 1],
                                         scale=pp[:, o, C1i:C1i + 1])

                # token mixer LN stats via fp8 DR on x8 (ones=1; /D at readout)
                sq_t = mid.tile([P, DO, W], FP8, tag="sq8", name=f"sq_{it}")
                nc.scalar.activation(sq_t, x8_t[:, :, 0:W], AF.Square)
                ps_m = pstC.tile([P, W], F32, tag="psm", name=f"psm_{it}")
                stats_mm8(ps_m, x8_t[:, :, 0:W], W)
                ps_s = pstC.tile([P, W], F32, tag="pss", name=f"pss_{it}")
                stats_mm8(ps_s, sq_t, W)
                m_sb = smC.tile([P, W], BF16, tag="msb", name=f"msb_{it}")
                nc.scalar.activation(m_sb, ps_m, AF.Copy, scale=1.0 / D)
                var = smC.tile([P, W], F32, tag="var", name=f"var_{it}")
                nc.scalar.activation(var, ps_m, AF.Square, scale=1.0 / D)
                nc.vector.scalar_tensor_tensor(var, ps_s, 1.0 / D, var,
                                               OP.mult, OP.subtract)
                nc.scalar.activation(var, var, AF.Sqrt, bias=eps_ln[:, 0:1])
                nc.vector.reciprocal_approx_fast(out=var, in_=var)
                rstd = smC.tile([P, W], BF16, tag="rstd", name=f"rstd_{it}")
                nc.vector.tensor_copy(rstd, var)
                # xm = (x - m) * rstd  (tm gamma folded into conv1 taps)
                xm_t = mid.tile([P, DO, W], BF16, tag="tokC", name=f"xm_{it}")
                nc.vector.tensor_sub(xm_t, x_t,
                                     m_sb[:, None, :].broadcast_to([P, DO, W]))
                nc.vector.tensor_mul(xm_t, xm_t,
                                     rstd[:, None, :].broadcast_to([P, DO, W]))
                if use_tmb:
                    for o in range(DO):
                        nc.vector.tensor_scalar_add(xm_t[:, o, :], xm_t[:, o, :],
                                                    pp[:, o, TMB:TMB + 1])
                # conv1: t_s[k] = conv1(xm)[k+1], k in [0, W-2)
                t_t = mid.tile([P, DO, W - 2], BF16, tag="tokD", name=f"t_{it}")
                for o in range(DO):
                    nc.scalar.activation(t_t[:, o, :], xm_t[:, o, 1:W - 1],
                                         AF.Identity, bias=pp[:, o, TCB1:TCB1 + 1],
                                         scale=pp[:, o, T1:T1 + 1])
                for o in range(DO):
                    nc.vector.scalar_tensor_tensor(t_t[:, o, :], xm_t[:, o, 0:W - 2],
                                                   pp[:, o, T0:T0 + 1],
                                                   t_t[:, o, :], OP.mult, OP.add)
                for o in range(DO):
                    nc.vector.scalar_tensor_tensor(t_t[:, o, :], xm_t[:, o, 2:W],
                                                   pp[:, o, T2:T2 + 1],
                                                   t_t[:, o, :], OP.mult, OP.add)
                t2_t = pipe.tile([P, DO, W - 2], BF16, tag="t2", name=f"t2_{it}")
                nc.scalar.activation(t2_t, t_t, AF.Gelu)
                if it == 0:
                    nc.vector.memset(t2_t[:, :, 0:1], 0.0)
                if it == NTILES - 1:
                    nc.vector.memset(t2_t[:, :, W - 3:W - 2], 0.0)
                return x_t, h1_t, t2_t

            def c_back(it, tiles):
                n0 = it * NT
                x_t, h1_t, t2_t = tiles
                for dc in range(DO):
                    # token-mixer conv2 taps on DVE (all-bf16 SBUF ops): phase
                    # C is PE-bound with DVE slack, so the three diagonal
                    # matmuls these replace were pure PE cost.
                    tm = smC.tile([P, NT], BF16, tag="tmconv", name=f"tm_{it}_{dc}")
                    nc.vector.tensor_scalar_mul(tm, t2_t[:, dc, 0:NT],
                                                pp[:, dc, U0:U0 + 1])
                    nc.vector.scalar_tensor_tensor(tm, t2_t[:, dc, 1:NT + 1],
                                                   pp[:, dc, U1:U1 + 1],
                                                   tm, OP.mult, OP.add)
                    nc.vector.scalar_tensor_tensor(tm, t2_t[:, dc, 2:NT + 2],
                                                   pp[:, dc, U2:U2 + 1],
                                                   tm, OP.mult, OP.add)
                    # own tag: MLP2's consumer (the DVE acc-add) lands late in
                    # the DVE queue; sharing slots would WAR-block MLP1/B.
                    ps_h = psC.tile([P, NT], F32, tag="psh2", name=f"psh2_{it}_{dc}")
                    for kc in range(DO):
                        nc.tensor.matmul(ps_h, w2_sb[:, kc, dc * P:(dc + 1) * P],
                                         h1_t[:, kc, :],
                                         start=(kc == 0), stop=False)
                    # diffusion conv side taps as diagonal matmuls
                    nc.tensor.matmul(ps_h, diagsb[:, 0, dc, :],
                                     x_t[:, dc, 1:NT + 1], start=False, stop=False)
                    nc.tensor.matmul(ps_h, diagsb[:, 1, dc, :],
                                     x_t[:, dc, 3:NT + 3], start=False, stop=True)
                    nc.vector.tensor_add(tm, tm, ps_h)
                    nc.vector.tensor_add(acc_full[:, dc, n0:n0 + NT],
                                         acc_full[:, dc, n0:n0 + NT], tm)

            n_b = len(b_groups) - DEFER_B
            per_tile = (n_b + NTILES - 1) // NTILES
            pend = {}
            for it in range(NTILES):
                if it + 1 < NTILES:
                    load_xc(it + 1)
                for g in b_groups[it * per_tile:min((it + 1) * per_tile, n_b)]:
                    b_group(*g, psC, "psh")
                if it == 0:
                    pend[0] = c_front(0)
                if it + 1 < NTILES:
                    pend[it + 1] = c_front(it + 1)
                c_back(it, pend.pop(it))

        # ---------------- Phase D: attention + LN1 + FFN + LN2 ----------------
        with ExitStack() as ph:
            ioD = ph.enter_context(tc.tile_pool(name="ioD", bufs=3))
            qp_tiles = {}

            def load_qp(it):
                t = ioD.tile([P, DO, NT], FP8, tag="qpD", name=f"qp_{it}")
                nc.sync.dma_start(t, qp_r[:, :, it * NT:(it + 1) * NT])
                qp_tiles[it] = t

            # qp(0) queued ahead of the FFN weight DMAs so the first
            # numerator matmuls don't wait for the weights
            load_qp(0)
            wpoolD = ph.enter_context(tc.tile_pool(name="wD", bufs=1))
            f1_sb = wpoolD.tile([P, DO, D], BF16, tag="f1")
            nc.sync.dma_start(f1_sb, wr["f1T"])
            f2_sb = wpoolD.tile([P, DO, D], FP8, tag="f2")
            nc.sync.dma_start(f2_sb, wr["f2T"])
            # y1(t) overlaps y1(t+1) (bufs=2); the other D intermediates are
            # dead before their next-tile reincarnation (bufs=1).
            midD2 = ph.enter_context(tc.tile_pool(name="midD2", bufs=2))
            midD = ph.enter_context(tc.tile_pool(name="midD", bufs=1))
            sm = ph.enter_context(tc.tile_pool(name="smD", bufs=2))
            ps = ph.enter_context(tc.tile_pool(name="psD", bufs=2, space="PSUM"))
            psf_pool = ph.enter_context(tc.tile_pool(name="psfD", bufs=2, space="PSUM"))
            pst = ph.enter_context(tc.tile_pool(name="pstD", bufs=1, space="PSUM"))

            def d_norm(it):
                """norm row + reciprocal chain (DVE) for tile it."""
                if it not in qp_tiles:
                    load_qp(it)
                qp_t = qp_tiles[it]
                if it + 1 < NTILES:
                    load_qp(it + 1)
                ps_n = pst.tile([P, NT], F32, tag="psnrep", name=f"psn_{it}")
                for kc in range(DO):
                    nc.tensor.matmul(ps_n[0:1, :], ksum_sb[:, kc, :],
                                     qp_t[:, kc, :],
                                     start=(kc == 0), stop=(kc == DO - 1))
                nr = sm.tile([1, NT], F32, tag="nrD", name=f"nr_{it}")
                nc.vector.tensor_scalar_add(nr, ps_n[0:1, :], 1e-6)
                rr = sm.tile([1, NT], F32, tag="rrD", name=f"rr_{it}")
                nc.vector.reciprocal_approx_fast(out=rr, in_=nr)
                rrb = sm.tile([1, NT], BF16, tag="rrbD", name=f"rrb_{it}")
                nc.vector.tensor_copy(rrb, rr)
                return rrb

            def d_rep(it, rrb):
                """rep = C1/norm broadcast (PE+ACT part)."""
                ps_rep = pst.tile([P, NT], F32, tag="psnrep", name=f"psrep_{it}")
                nc.tensor.matmul(ps_rep, repc_row[0:1, :], rrb, start=True,
                                 stop=True)
                rep_sb = sm.tile([P, NT], BF16, tag="repsb", name=f"rep_{it}")
                nc.scalar.activation(rep_sb, ps_rep, AF.Copy)
                return rep_sb

            def d_qpscale(it, rep_sb):
                """qp *= rep: the ~4us DVE op, emitted at the iteration tail
                so it never sits ahead of y1 in the DVE queue (norm/rep run
                two tiles ahead, so there is a full iteration of slack)."""
                qp_t = qp_tiles[it]
                nc.vector.tensor_mul(qp_t, qp_t,
                                     rep_sb[:, None, :].broadcast_to([P, DO, NT]))

            def d_num(it):
                """numerator chunks; squares on GPSIMD (SBUF->SBUF)."""
                n0 = it * NT
                qp_t = qp_tiles.pop(it)
                acc_t = acc_full[:, :, n0:n0 + NT]
                sq_t = midD.tile([P, DO, NT], FP8, tag="sqD8", name=f"sqD_{it}")
                for ec in range(DO):
                    ps_u = ps.tile([P, NT], F32, tag="psnum", name=f"psnum_{it}_{ec}")
                    for kp in range(0, DO, 2):
                        nc.tensor.matmul(ps_u, kv_sb[:, kp:kp + 2, ec * P:(ec + 1) * P],
                                         qp_t[:, kp:kp + 2, :],
                                         start=(kp == 0), stop=(kp == DO - 2),
                                         perf_mode=DR)
                    nc.vector.scalar_tensor_tensor(acc_t[:, ec, :], ps_u,
                                                   1.0 / (KV_SCALE * C1),
                                                   acc_t[:, ec, :], OP.mult, OP.add)
                    nc.scalar.activation(sq_t[:, ec, :], acc_t[:, ec, :],
                                         AF.Square)
                return acc_t, sq_t

            def d_stats(y_t, sq_t, mtag, stag, name):
                """LN stats in one batch (streamed-per-chunk stats make each
                stats matmul a hard PE->DVE/GPSIMD sync point)."""
                ps_m = pst.tile([P, NT], F32, tag=mtag, name=f"psm{name}")
                ps_s = pst.tile([P, NT], F32, tag=stag, name=f"pss{name}")
                stats_mm(ps_m, onesD_bf, y_t, NT)
                stats_mm8(ps_s, sq_t, NT)
                return ps_m, ps_s

            def ln_pre(ps_m, ps_s, kind, name):
                """mean copy + var (without the Rsqrt): same-ACT-table ops."""
                m_sb = sm.tile([P, NT], BF16, tag=f"m{kind}", name=f"m{name}")
                nc.scalar.activation(m_sb, ps_m, AF.Copy)
                var = sm.tile([P, NT], F32, tag=f"v{kind}", name=f"v{name}")
                nc.scalar.activation(var, ps_m, AF.Square)
                nc.vector.scalar_tensor_tensor(var, ps_s, 1.0 / D, var,
                                               OP.mult, OP.subtract)
                return m_sb, var

            def ln_rstd(var, kind, name):
                nc.scalar.activation(var, var, AF.Sqrt, bias=eps_ln[:, 0:1])
                nc.vector.reciprocal_approx_fast(out=var, in_=var)
                rstd = sm.tile([P, NT], BF16, tag=f"r{kind}", name=f"r{name}")
                nc.vector.tensor_copy(rstd, var)
                return rstd

            def d_y1(it, acc_t, m1_sb, rstd1):
                y1_t = midD2.tile([P, DO, NT], BF16, tag="y1", name=f"y1_{it}")
                for kc in range(DO):
                    nc.vector.tensor_sub(y1_t[:, kc, :], acc_t[:, kc, :], m1_sb)
                    nc.vector.tensor_mul(y1_t[:, kc, :], y1_t[:, kc, :], rstd1)
                    if use_n1b:
                        nc.vector.tensor_scalar_add(y1_t[:, kc, :], y1_t[:, kc, :],
                                                    pp[:, kc, N1B:N1B + 1])
                return y1_t

            def d_ffn1(it, y1_t, dcs):
                """FFN1 (bf16) for output chunks dcs; gelu writes fh fp8."""
                if it not in ffn1_fh:
                    ffn1_fh[it] = midD.tile([P, DO, NT], FP8, tag="f1h",
                                            name=f"f1h_{it}")
                fh_t = ffn1_fh[it]
                for dc in dcs:
                    ps_f = psf_pool.tile([P, NT], F32, tag="psf",
                                         name=f"psf1_{it}_{dc}")
                    for kc in range(DO):
                        nc.tensor.matmul(ps_f, f1_sb[:, kc, dc * P:(dc + 1) * P],
                                         y1_t[:, kc, :],
                                         start=(kc == 0), stop=(kc == DO - 1))
                    nc.scalar.activation(fh_t[:, dc, :], ps_f, AF.Gelu,
                                         bias=pp[:, dc, FFB1:FFB1 + 1])
                return fh_t

            def d_ffn2(it, y1_t, fh_t):
                """FFN2 in fp8 DR (f2 x16) + residual.  The psum scale runs
                on ACT and the residual add on DVE (cheap all-bf16 op);
                squares on GPSIMD."""
                y2_t = midD.tile([P, DO, NT], BF16, tag="y2", name=f"y2_{it}")
                sq2_t = midD.tile([P, DO, NT], FP8, tag="sq28", name=f"sq2_{it}")
                for dc in range(DO):
                    ps_f = psf_pool.tile([P, NT], F32, tag="psf",
                                         name=f"psf2_{it}_{dc}")
                    for kp in range(0, DO, 2):
                        nc.tensor.matmul(ps_f, f2_sb[:, kp:kp + 2, dc * P:(dc + 1) * P],
                                         fh_t[:, kp:kp + 2, :],
                                         start=(kp == 0), stop=(kp == DO - 2),
                                         perf_mode=DR)
                    y2tmp = sm.tile([P, NT], BF16, tag="y2t", name=f"y2t_{it}_{dc}")
                    nc.scalar.activation(y2tmp, ps_f, AF.Copy, scale=1.0 / W16)
                    if use_n1g:
                        # y2 = y1*g + f2(h)/16 (residual gamma re-applied)
                        nc.vector.scalar_tensor_tensor(y2_t[:, dc, :],
                                                       y1_t[:, dc, :],
                                                       pp[:, dc, N1G:N1G + 1],
                                                       y2tmp, OP.mult, OP.add)
                    else:
                        nc.vector.tensor_add(y2_t[:, dc, :], y2tmp,
                                             y1_t[:, dc, :])
                    nc.scalar.activation(sq2_t[:, dc, :], y2_t[:, dc, :],
                                         AF.Square)
                return y2_t, sq2_t

            def d_out(it, y2_t, m2_sb, rstd2):
                n0 = it * NT
                yo_t = midD.tile([P, DO, NT], BF16, tag="yo", name=f"yo_{it}")
                # last tile: two half-width DMAs so the final drain pipelines
                nsplit = 2 if it == NTILES - 1 else 1
                cw = NT // nsplit
                for ci in range(nsplit):
                    cs = slice(ci * cw, (ci + 1) * cw)
                    for dc in range(DO):
                        yo = yo_t[:, dc, cs]
                        nc.vector.tensor_sub(yo, y2_t[:, dc, cs], m2_sb[:, cs])
                        if use_n2g:
                            nc.vector.scalar_tensor_tensor(yo, yo,
                                                           pp[:, dc, N2G:N2G + 1],
                                                           rstd2[:, cs],
                                                           OP.mult, OP.mult)
                        else:
                            nc.vector.tensor_mul(yo, yo, rstd2[:, cs])
                        if use_n2b:
                            nc.vector.tensor_scalar_add(yo, yo,
                                                        pp[:, dc, N2B:N2B + 1])
                    nc.sync.dma_start(yT[:, :, n0 + ci * cw:n0 + (ci + 1) * cw],
                                      yo_t[:, :, cs])

            ffn1_fh = {}
            # Prologue: norm+rep two tiles deep (the GPSIMD qp-scale takes
            # ~8us and the numerator gates on it), deferred B groups to keep
            # PE fed meanwhile, then tile 0's numerator + LN1.
            d_qpscale(0, d_rep(0, d_norm(0)))
            for g in b_groups[len(b_groups) - DEFER_B:]:
                b_group(*g, ps, "psnum")
            kvres_stack.close()
            d_qpscale(1, d_rep(1, d_norm(1)))
            acc0, sq0 = d_num(0)
            st_m, st_s = d_stats(acc0, sq0, "psm1", "pss1", "1_0")
            m1_sb, var1 = ln_pre(st_m, st_s, "1", "1_0")
            rstd1 = ln_rstd(var1, "1", "1_0")
            y1_cur = d_y1(0, acc0, m1_sb, rstd1)
            for it in range(NTILES):
                d_ffn1(it, y1_cur, range(DO))
                Tn = d_num(it + 1) if it + 1 < NTILES else None
                rep_n = d_rep(it + 2, d_norm(it + 2)) if it + 2 < NTILES else None
                y2_t, sq2_t = d_ffn2(it, y1_cur, ffn1_fh.pop(it))
                # LN chains batched by ACT table: both pre-chains (copy +
                # square), then both Sqrts, then the applies.
                if Tn is not None:
                    acc_n, sq_n = Tn
                    m1n_ps, s1n_ps = d_stats(acc_n, sq_n, "psm1", "pss1",
                                             f"1_{it + 1}")
                m2_ps, s2_ps = d_stats(y2_t, sq2_t, "psnrep", "pss2", f"2_{it}")
                if Tn is not None:
                    m1n, var1n = ln_pre(m1n_ps, s1n_ps, "1", f"1_{it + 1}")
                m2_sb, var2 = ln_pre(m2_ps, s2_ps, "2", f"2_{it}")
                if Tn is not None:
                    rstd1n = ln_rstd(var1n, "1", f"1_{it + 1}")
                    y1_cur = d_y1(it + 1, acc_n, m1n, rstd1n)
                rstd2 = ln_rstd(var2, "2", f"2_{it}")
                d_out(it, y2_t, m2_sb, rstd2)
                if rep_n is not None:
                    d_qpscale(it + 2, rep_n)

    nc.compile()
    return nc


def make_in_maps(inputs, n_cores=8):
    """Host-side preprocessing: fold constants, transpose, cast, shard."""
    x = np.asarray(inputs["x"], np.float32)
    B, N, D_ = x.shape
    dt = float(np.asarray(inputs["delta_t"]))

    def g(k):
        return np.asarray(inputs[k], np.float32)

    diff_w, diff_b = g("diff_w"), g("diff_b")
    tm_w1, tm_cb1 = g("tm_w1"), g("tm_cb1")
    tm_w2, tm_cb2 = g("tm_w2"), g("tm_cb2")
    tm_g = g("tm_g")
    n1_g, n2_g = g("n1_g"), g("n2_g")

    pp = np.zeros((P, DO, NPARAM), np.float32)

    def put(i, v):
        pp[:, :, i] = v.reshape(DO, P).T

    put(C0, dt * diff_w[:, 0, 0])
    put(C1i, dt * diff_w[:, 0, 1] + (1.0 - dt))
    put(C2, dt * diff_w[:, 0, 2])
    put(CB, dt * diff_b + g("lu_b2") + tm_cb2)
    # token-mixer gamma folded into the conv1 taps
    put(T0, tm_w1[:, 0, 0] * tm_g)
    put(T1, tm_w1[:, 0, 1] * tm_g)
    put(T2, tm_w1[:, 0, 2] * tm_g)
    put(TCB1, tm_cb1)
    put(U0, tm_w2[:, 0, 0])
    put(U1, tm_w2[:, 0, 1])
    put(U2, tm_w2[:, 0, 2])
    put(TMG, tm_g)
    put(TMB, g("tm_beta"))
    put(N1G, n1_g)
    put(N1B, g("n1_b"))
    put(N2G, n2_g)
    put(N2B, g("n2_b"))
    put(LUB1, g("lu_b1"))
    put(FFB1, g("ff_b1"))
    put(FFB2, g("ff_b2"))

    # diags (fp8): token-mixer conv2 taps; diagsb (bf16): diffusion side taps
    diags = np.zeros((P, 3, DO, P), np.float32)
    diagsb = np.zeros((P, 2, DO, P), np.float32)
    idx = np.arange(P)
    for tap in range(3):
        for dc in range(DO):
            diags[idx, tap, dc, idx] = tm_w2[dc * P + idx, 0, tap]
    for dc in range(DO):
        diagsb[idx, 0, dc, idx] = dt * diff_w[dc * P + idx, 0, 0]
        diagsb[idx, 1, dc, idx] = dt * diff_w[dc * P + idx, 0, 2]
    diags = np.clip(diags, -240, 240).astype(FP8_NP)
    diagsb = diagsb.astype(BF16_NP)

    rows = np.zeros((1, 3 * D), np.float32)
    rows[0, 0:D] = g("bq")
    rows[0, D:2 * D] = g("bk")
    rows[0, 2 * D:3 * D] = g("bv")
    rows = rows.astype(BF16_NP)

    use_n1g = bool(np.any(n1_g != 1.0))
    use_n2g = bool(np.any(n2_g != 1.0))

    wt = {}
    # The local-MLP branch stays fully bf16 (fp8 there dominated the
    # error budget)
    wt["w1T"] = np.ascontiguousarray(g("lu_w1").T).astype(BF16_NP)
    wt["w2T"] = np.ascontiguousarray(g("lu_w2").T).astype(BF16_NP)
    # n1 gamma folded into ff_w1 input rows (when gamma != 1)
    f1 = g("ff_w1")
    if use_n1g:
        f1 = f1 * n1_g[None, :]
    wt["f1T"] = np.ascontiguousarray(f1.T).astype(BF16_NP)
    # FFN2 weights shipped fp8 pre-scaled x16 (consumer applies 1/16)
    wt["f2T"] = np.ascontiguousarray(
        np.clip(g("ff_w2").T * W16, -240, 240)).astype(FP8_NP)
    for name, key in (("wqT", "wq"), ("wkT", "wk"), ("wvT", "wv")):
        wt[name] = np.ascontiguousarray(
            np.clip(g(key).T, -240, 240)).astype(FP8_NP)

    xT = np.ascontiguousarray(x.transpose(0, 2, 1)).astype(BF16_NP)
    x8 = np.clip(xT.astype(np.float32), -240, 240).astype(FP8_NP)

    flags = dict(
        use_bq=bool(np.any(g("bq"))),
        use_bk=bool(np.any(g("bk"))),
        use_bv=bool(np.any(g("bv"))),
        use_tmb=bool(np.any(g("tm_beta"))),
        use_n1b=bool(np.any(g("n1_b"))),
        use_n2b=bool(np.any(g("n2_b"))),
        use_n1g=use_n1g,
        use_n2g=use_n2g,
    )

    shared = {**wt, "pp": pp, "rows": rows, "diags": diags,
              "diagsb": diagsb}
    in_maps = [{**shared, "x_T": xT[b], "x8": x8[b]} for b in range(B)]
    return in_maps, flags, (B, N)


_NC_CACHE = {}


def kernel(**inputs):
    in_maps, flags, (B, N) = make_in_maps(inputs)
    key = (N, tuple(sorted(flags.items())))
    if key not in _NC_CACHE:
        _NC_CACHE[key] = build_nc(N=N, NT=512, **flags)
    nc = _NC_CACHE[key]
    res = run_bass_kernel_spmd(nc, in_maps, list(range(B)))
    y = np.stack([res.results[b]["y_T"] for b in range(B)])
    return np.ascontiguousarray(y.transpose(0, 2, 1)).astype(np.float32)
